# revision 12
# baseline (speedup 1.0000x reference)
"""BiMapGeo forward on 8 NeuronCores (TRN2, Bass/Tile).

P[b,o] = sum_c W[o,c]^T @ x[b,c] @ W[o,c]
  x: (256, 8, 128, 128) fp32 (symmetric in last two dims)
  W: (16, 8, 128, 64) fp32
  P: (256, 16, 64, 64) fp32

Sharding: data-parallel over batch (32 per core), W replicated.

Per-core structure, all-bf16 PE path, chunks of CH=4 batches:
  mm1: M1[b,c] = x[b,c] @ W[:,c]      (stationary=x_bf16[b,c] via symmetry,
                                       moving=W[:,c] as [128,1024] in 2x512)
  evict: PSUM fp32 -> SBUF bf16       (whole tiles, alternating DVE/Act)
  mm2: P[b-chunk,o] += W[o,c]^T @ M1  (N=256 = 4 batches x 64 q, o-pairs
                                       col-tiled, accumulate c in PSUM)
  evict P -> SBUF (alternating) -> one DMA per o-pair.
  mm2 of chunk g-1 is interleaved 2 steps per mm1 step of chunk g so PE,
  DVE and Act all stay busy concurrently.
"""

import numpy as np
from contextlib import ExitStack

import concourse.bacc as bacc
import concourse.tile as tile
from concourse import mybir

B_TOT, HI, HO, NI, NO = 256, 8, 16, 128, 64
NCORES = 8
B = B_TOT // NCORES  # 32 batches per core
CH = 4               # batches per chunk
NCH = B // CH        # 8 chunks
OQ = HO * NO         # 1024

F32 = mybir.dt.float32
BF16 = mybir.dt.bfloat16

_NC_CACHE = {}


def build_nc(loop_iters: int = 1):
    nc = bacc.Bacc("TRN2", target_bir_lowering=False, debug=False)

    x_in = nc.dram_tensor("x", [B, HI, NI, NI], F32, kind="ExternalInput")
    w_in = nc.dram_tensor("W", [HO, HI, NI, NO], F32, kind="ExternalInput")
    p_out = nc.dram_tensor("P", [B, HO, NO, NO], F32, kind="ExternalOutput")

    with tile.TileContext(nc) as tc, ExitStack() as ctx:
        const = ctx.enter_context(tc.tile_pool(name="const", bufs=1))
        wstage = ctx.enter_context(tc.tile_pool(name="wstage", bufs=2))
        xstage = ctx.enter_context(tc.tile_pool(name="xstage", bufs=3))
        xbf = ctx.enter_context(tc.tile_pool(name="xbf", bufs=2))
        m1sb = ctx.enter_context(tc.tile_pool(name="m1sb", bufs=2))
        psb = ctx.enter_context(tc.tile_pool(name="psb", bufs=3))
        m1ps = ctx.enter_context(tc.tile_pool(name="m1ps", bufs=3, space="PSUM"))
        pps = ctx.enter_context(tc.tile_pool(name="pps", bufs=2, space="PSUM"))

        # W resident in SBUF as [i/j (128), c, o, q] bf16: serves as mm1
        # moving operand ([j, c, (o q)]) and mm2 stationary ([i, c, o, p]).
        w_bf = const.tile([NI, HI, HO, NO], BF16, tag="w_bf")
        for c in range(HI):
            w_st = wstage.tile([NI, HO, NO], F32, tag="wst", name=f"wst{c}")
            nc.scalar.dma_start(out=w_st[:], in_=w_in[:, c, :, :].transpose([1, 0, 2]))
            nc.vector.tensor_copy(w_bf[:, c], w_st[:])

        def emit_body():
            emit_iter(nc, x_in, p_out, w_bf, xstage, xbf, m1sb, psb, m1ps, pps)

        if loop_iters > 1:
            ET = mybir.EngineType
            with tc.For_i(0, loop_iters, 1, hint_engines=(ET.PE, ET.DVE, ET.Activation, ET.SP), staggered_reset=True):
                emit_body()
        else:
            emit_body()
    nc.finalize()
    return nc


def emit_iter(nc, x_in, p_out, w_bf, xstage, xbf, m1sb, psb, m1ps, pps):
    def x_load(g):
        # x chunk as [i(128), b, c, j] bf16; by symmetry also [j, b, c, i].
        x_t = xbf.tile([NI, CH, HI, NI], BF16, tag="xbf", name=f"xbf{g}")
        for b in range(CH):
            x_sb = xstage.tile([NI, HI, NI], F32, tag="xst", name=f"xst{g}b{b}")
            nc.sync.dma_start(out=x_sb[:], in_=x_in[g * CH + b].transpose([1, 0, 2]))
            nc.gpsimd.tensor_copy(x_t[:, b], x_sb[:])
        return x_t

    def mm1_steps(g, x_t, m1_t):
        # 32 steps: per (b, c) one stationary load + 2 MMs N=512, then a
        # whole-tile eviction on DVE or Act (alternating).
        for b in range(CH):
            for c in range(HI):
                ps = m1ps.tile([NI, OQ], F32, tag="m1ps")
                for h in range(2):
                    for ph in range(2):
                        nc.tensor.matmul(
                            ps[ph * 64 : (ph + 1) * 64, h * 512 : (h + 1) * 512],
                            x_t[:, b, c, ph * 64 : (ph + 1) * 64],
                            w_bf[:, c, h * 8 : (h + 1) * 8, :],
                            start=True,
                            stop=True,
                            tile_position=(0, ph * 64),
                            skip_group_check=True,
                        )
                if (b * HI + c) % 2 == 0:
                    nc.vector.tensor_copy(m1_t[:, b, c, :], ps[:])
                else:
                    nc.scalar.copy(m1_t[:, b, c, :], ps[:])
                yield

    def mm2_steps(g, m1_t):
        # 8 o-pairs x 8 c = 64 steps of 2 MMs (N=256), PSUM-accumulated
        for t in range(8):
            pt = pps.tile([NI, CH, NO], F32, tag="pps", name=f"pps_g{g}t{t}")
            for c in range(HI):
                for ph in range(2):
                    o = 2 * t + ph
                    nc.tensor.matmul(
                        pt[ph * 64 : (ph + 1) * 64, :, :],
                        w_bf[:, c, o, :],
                        m1_t[:, :, c, o * 64 : (o + 1) * 64],
                        start=(c == 0),
                        stop=(c == HI - 1),
                        tile_position=(0, ph * 64),
                        skip_group_check=True,
                    )
                yield
            p_sb = psb.tile([NI, CH, NO], F32, tag="psb", name=f"psb_g{g}t{t}")
            if t % 2 == 0:
                nc.vector.tensor_copy(p_sb[:], pt[:])
            else:
                nc.scalar.copy(p_sb[:], pt[:])
            nc.gpsimd.dma_start(
                out=p_out[g * CH : (g + 1) * CH, 2 * t : 2 * t + 2].rearrange(
                    "b o p q -> (o p) b q"
                ),
                in_=p_sb[:],
            )

    x_tiles = {0: x_load(0)}
    prev_mm2 = None
    for g in range(NCH):
        if g + 1 < NCH:
            x_tiles[g + 1] = x_load(g + 1)
        m1_t = m1sb.tile([NI, CH, HI, OQ], BF16, tag="m1sb", name=f"m1_{g}")
        for _ in mm1_steps(g, x_tiles.pop(g), m1_t):
            if prev_mm2 is not None:
                next(prev_mm2, None)
                next(prev_mm2, None)
        if prev_mm2 is not None:
            for _ in prev_mm2:
                pass
        prev_mm2 = mm2_steps(g, m1_t)
    for _ in prev_mm2:
        pass


def kernel(x: np.ndarray, W: np.ndarray) -> np.ndarray:
    from concourse.bass_utils import run_bass_kernel_spmd

    x = np.ascontiguousarray(x, dtype=np.float32)
    W = np.ascontiguousarray(W, dtype=np.float32)

    if "nc" not in _NC_CACHE:
        _NC_CACHE["nc"] = build_nc()
    nc = _NC_CACHE["nc"]

    in_maps = [
        {"x": x[i * B : (i + 1) * B], "W": W} for i in range(NCORES)
    ]
    res = run_bass_kernel_spmd(nc, in_maps, list(range(NCORES)))
    out = np.concatenate([res.results[i]["P"] for i in range(NCORES)], axis=0)
    return out
